# revision 35
# baseline (speedup 1.0000x reference)
"""Trainium2 Bass kernel for nn_AqSolModel (GNN message passing), 8 NeuronCores.

Strategy (v2):
- Node-sharded: core c owns 6250 nodes in 50 blocks x 128 slots, split into
  two fixed chunks (A: blocks 0-23, B: blocks 24-49; node->chunk fixed by
  node index, so edge source chunks are known before packing).
- Activation pool in fp8 is exchanged per layer with TWO AllGathers (one per
  chunk): AG_A fires mid-layer (hidden under compute of chunk-B groups),
  AG_B at layer end overlaps the next layer's chunk-A gathers.
- Pool rows are p-major within a chunk so per-group bounce writes coalesce.
- Per-edge source rows fetched with dma_gather (fp8, 512B rows); segment-sum
  via matmuls against 0/1 selection tiles M (fp8, RESIDENT in SBUF, loaded
  once) + identity matmul for the self loop.
- All BatchNorms folded on host: activations are stored pre-scaled by the
  next layer's BN_in gain (folded into dense2 weights); the BN_in bias enters
  dense1 as a rank-1 (W1^T c)·deg^T term via a K=1 matmul; dense biases are
  K=1 matmuls against constant rows.
- Mean-pool via per-block selection matmul into a per-core graph window;
  windows AllGathered; small dense head runs redundantly on all cores.
"""
import sys
sys.path.insert(0, "/opt/trn_rl_repo")

import numpy as np
import ml_dtypes

BF16 = ml_dtypes.bfloat16
FP8 = ml_dtypes.float8_e4m3

N_NODES, N_EDGES, N_FEAT, HID, HID1, N_GRAPHS, N_CONV, N_LIN = (
    50000, 150000, 128, 512, 320, 2048, 4, 3)
EPS = 1e-5
NC_ = 8
SHARD = N_NODES // NC_          # 6250
BLKS = 50
SLOTS = BLKS * 128              # 6400
ABLKS, BBLKS = 28, 22           # chunk A / chunk B blocks
ASLOTS, BSLOTS = ABLKS * 128, BBLKS * 128   # 3584 / 2816
ACHN = 3500                     # nodes of each core assigned to chunk A
POOL_A, POOL_B = NC_ * ASLOTS, NC_ * BSLOTS  # 28672 / 22528 (both < 32768)
PG = 384                        # pooling window width (3*128)
GRP = 4                         # blocks per dense group
F1P = 384                       # HID1 padded to 3*128

# ---------------------------------------------------------------- host planning


def _pack2(degA, degB, nblocks, capA, capB):
    """Assign len(degA) nodes to nblocks blocks s.t. per block:
    count <= 128, sum(degA) <= capA, sum(degB) <= capB.
    Returns block index per node or None."""
    n = len(degA)
    tot = degA + degB
    order = np.argsort(-tot)
    cnt = np.zeros(nblocks, np.int32)
    la = np.zeros(nblocks, np.int64)
    lb = np.zeros(nblocks, np.int64)
    assign = np.full(n, -1, np.int32)
    for node in order:
        a, b = degA[node], degB[node]
        ok = (cnt < 128) & (la + a <= capA) & (lb + b <= capB)
        if not ok.any():
            return None
        cand = np.nonzero(ok)[0]
        j = cand[np.argmin(la[cand] + lb[cand])]
        assign[node] = j
        cnt[j] += 1
        la[j] += a
        lb[j] += b
    return assign


def build_plan(edge_index, batch):
    src = edge_index[0].astype(np.int64)
    dst = edge_index[1].astype(np.int64)
    core_of = np.minimum(np.arange(N_NODES) // SHARD, NC_ - 1)
    within = np.arange(N_NODES) - core_of * SHARD
    chunk_of = (within >= ACHN).astype(np.int64)   # 0=A, 1=B (fixed!)

    src_chunk = chunk_of[src]
    degA_n = np.bincount(dst[src_chunk == 0], minlength=N_NODES)
    degB_n = np.bincount(dst[src_chunk == 1], minlength=N_NODES)

    slot_of = np.zeros(N_NODES, np.int64)
    TL = TH = 2
    while True:
        ok = True
        for c in range(NC_):
            for ch, nb, boff in ((0, ABLKS, 0), (1, BBLKS, ABLKS)):
                nodes = np.nonzero((core_of == c) & (chunk_of == ch))[0]
                a = _pack2(degA_n[nodes], degB_n[nodes], nb,
                           TL * 128, TH * 128)
                if a is None:
                    ok = False
                    break
                blk = boff + a
                # slot position within block
                pos = np.zeros(len(nodes), np.int64)
                nxt = np.zeros(ABLKS + BBLKS, np.int64)
                for i in range(len(nodes)):
                    pos[i] = nxt[blk[i]]
                    nxt[blk[i]] += 1
                slot_of[nodes] = blk * 128 + pos
            if not ok:
                break
        if ok:
            break
        if TL <= TH:
            TL += 1
        else:
            TH += 1
    NT = TL + TH

    blk_of = slot_of // 128
    p_of = slot_of % 128
    # pool rows (p-major within chunk)
    rowA = core_of * ASLOTS + p_of * ABLKS + blk_of            # chunk==0
    rowB = core_of * BSLOTS + p_of * BBLKS + (blk_of - ABLKS)  # chunk==1
    pool_row = np.where(chunk_of == 0, rowA, rowB)

    dst_core = core_of[dst]
    dst_blk = blk_of[dst]
    dst_col = p_of[dst]

    idxA = np.zeros((NC_, BLKS, TL, 128), np.int16)
    idxB = np.zeros((NC_, BLKS, TH, 128), np.int16)
    m_all = np.zeros((NC_, BLKS, NT, 128, 128), np.float32)
    for c in range(NC_):
        sel = np.nonzero(dst_core == c)[0]
        b_of = dst_blk[sel]
        order = np.argsort(b_of, kind="stable")
        sel = sel[order]
        b_of = b_of[order]
        bounds = np.searchsorted(b_of, np.arange(BLKS + 1))
        for b in range(BLKS):
            es = sel[bounds[b]:bounds[b + 1]]
            ea = es[src_chunk[es] == 0]
            eb = es[src_chunk[es] == 1]
            assert len(ea) <= TL * 128 and len(eb) <= TH * 128, (c, b)
            for t0, eset, rows, idx_arr, ntile in (
                    (0, ea, pool_row, idxA, TL),
                    (TL, eb, pool_row, idxB, TH)):
                t = np.arange(len(eset)) // 128
                r = np.arange(len(eset)) % 128
                if idx_arr is idxA:
                    idxA[c, b, t, r] = rows[src[eset]].astype(np.int16)
                else:
                    idxB[c, b, t, r] = rows[src[eset]].astype(np.int16)
                m_all[c, b, t0 + t, r, dst_col[eset]] = 1.0

    deg = np.bincount(dst, minlength=N_NODES).astype(np.float32) + 1.0
    deg_slots = np.zeros((NC_, SLOTS), np.float32)
    deg_slots[core_of, slot_of] = deg

    # pooling windows
    cnt = np.bincount(batch, minlength=N_GRAPHS).astype(np.float32)
    inv_cnt = (1.0 / np.maximum(cnt, 1.0)).astype(np.float32)
    g_of = batch.astype(np.int64)
    wbase = np.zeros(NC_, np.int32)
    mpool = np.zeros((NC_, BLKS, 128, PG), np.float32)
    for c in range(NC_):
        nodes = np.arange(c * SHARD, (c + 1) * SHARD)
        gmin, gmax = g_of[nodes].min(), g_of[nodes].max()
        wb = min(max(0, (gmin + gmax + 1) // 2 - PG // 2), N_GRAPHS - PG)
        wb = min(wb, gmin)
        wb = max(wb, gmax - PG + 1)
        assert wb >= 0 and wb + PG <= N_GRAPHS and gmin >= wb and gmax < wb + PG
        wbase[c] = wb
        mpool[c, blk_of[nodes], p_of[nodes], g_of[nodes] - wb] = \
            inv_cnt[g_of[nodes]]

    return dict(slot_of=slot_of, core_of=core_of, chunk_of=chunk_of,
                pool_row=pool_row, TL=TL, TH=TH,
                idxA=idxA, idxB=idxB, M=m_all, deg=deg_slots,
                mpool=mpool, wbase=wbase)


def fold_params(p):
    """Fold all BNs. Returns per-layer dicts with:
    WA [F_in, F1], bA [F1], cvec [F1] (= WA^T c_in),
    WB [F1, 512] (scaled by gtil), bB [512] (gtil*b2 + ctil),
    gin [F_in] (only layer 0's is applied on host to x)."""
    raw = []
    for l in range(5):
        if l == 0:
            ing, inb, inm, inv = p['in_g1'], p['in_b1'], p['in_m1'], p['in_v1']
            wa, ba, wb, bb = p['w1a'], p['b1a'], p['w1b'], p['b1b']
            og, ob, om, ov = p['out_g1'], p['out_b1'], p['out_m1'], p['out_v1']
        else:
            i = l - 1
            ing, inb, inm, inv = (p['cin_g'][i], p['cin_b'][i],
                                  p['cin_m'][i], p['cin_v'][i])
            wa, ba, wb, bb = p['cwA'][i], p['cbA'][i], p['cwB'][i], p['cbB'][i]
            og, ob, om, ov = (p['cout_g'][i], p['cout_b'][i],
                              p['cout_m'][i], p['cout_v'][i])
        gin = np.asarray(ing, np.float64) / np.sqrt(
            np.asarray(inv, np.float64) + EPS)
        cin = np.asarray(inb, np.float64) - np.asarray(inm, np.float64) * gin
        gout = np.asarray(og, np.float64) / np.sqrt(
            np.asarray(ov, np.float64) + EPS)
        cout = np.asarray(ob, np.float64) - np.asarray(om, np.float64) * gout
        raw.append(dict(gin=gin, cin=cin, gout=gout, cout=cout,
                        wa=np.asarray(wa, np.float64),
                        ba=np.asarray(ba, np.float64),
                        wb=np.asarray(wb, np.float64),
                        bb=np.asarray(bb, np.float64)))
    out = []
    for l in range(5):
        r = raw[l]
        gnext = raw[l + 1]['gin'] if l < 4 else np.ones(HID)
        gtil = r['gout'] * gnext
        ctil = r['cout'] * gnext
        out.append(dict(
            WA=r['wa'].astype(np.float32),
            bA=r['ba'].astype(np.float32),
            cvec=(r['cin'] @ r['wa']).astype(np.float32),
            WB=(r['wb'] * gtil[None, :]).astype(np.float32),
            bB=(r['bb'] * gtil + ctil).astype(np.float32),
            gin=r['gin'].astype(np.float32)))
    return out


# ---------------------------------------------------------------- device build

def build_device(TL, TH):
    from concourse import bass, bacc, mybir, tile

    NT = TL + TH
    dt = mybir.dt
    nc = bacc.Bacc("TRN2", target_bir_lowering=False, debug=False,
                   enable_asserts=False, num_devices=NC_,
                   num_swdge_queues=4)

    def inp(name, shape, dtype):
        return nc.dram_tensor(name, shape, dtype, kind="ExternalInput")

    gxa_in = inp("gxa", [128, BLKS * TL * N_FEAT], dt.float8e4)
    gxb_in = inp("gxb", [128, BLKS * TH * N_FEAT], dt.float8e4)
    ux_in = inp("ux", [128, BLKS * N_FEAT], dt.bfloat16)
    gl_in = inp("gl", [128, BLKS * TL * 8], dt.int16)
    gh_in = inp("gh", [128, BLKS * TH * 8], dt.int16)
    m8_in = inp("m8", [128, BLKS * NT * 128], dt.float8e4)
    mp_in = inp("mp", [128, BLKS * PG], dt.bfloat16)
    deg_in = inp("deg", [1, SLOTS], dt.bfloat16)   # sliced per group
    cvec_in = inp("cvec", [1, 5 * 512], dt.bfloat16)
    b2row_in = inp("b2row", [1, 5 * 512], dt.bfloat16)
    onecol_in = inp("onecol", [1, 128], dt.bfloat16)
    pvec_in = inp("pvec", [128, 32], dt.float32)
    ident_in = inp("ident", [128, 128], dt.bfloat16)
    wa0_in = inp("wa0", [128, F1P], dt.bfloat16)
    wb0_in = inp("wb0", [128, 3 * 512], dt.bfloat16)
    wa_in = inp("wa", [4, 128, 4 * 512], dt.bfloat16)
    wb_in = inp("wb", [4, 128, 4 * 512], dt.bfloat16)
    lw_in = inp("lw", [3, 128, 4 * 512], dt.bfloat16)
    fw_in = inp("fw", [128, 4], dt.bfloat16)   # loaded in head phase
    out_ext = nc.dram_tensor("out", [N_GRAPHS, 1], dt.float32,
                             kind="ExternalOutput")

    groups = [list(range(g * GRP, min((g + 1) * GRP, BLKS)))
              for g in range((BLKS + GRP - 1) // GRP)]   # 12x4 + 1x2
    AG_GROUP = ABLKS // GRP - 1                          # AG_A after group 5

    with tile.TileContext(nc) as tc:
        import contextlib
        ctx = contextlib.ExitStack()
        with ctx:
            dram = ctx.enter_context(tc.tile_pool(name="dram", bufs=1,
                                                  space="DRAM"))
            const = ctx.enter_context(tc.tile_pool(name="const", bufs=1))

            pools_a = [dram.tile([POOL_A, HID], dt.float8e4,
                                 addr_space="Shared", name=f"pool_a{i}")
                       for i in range(4)]
            pools_b = [dram.tile([POOL_B, HID], dt.float8e4,
                                 addr_space="Shared", name=f"pool_b{i}")
                       for i in range(4)]
            bounces_a = [dram.tile([ASLOTS, HID], dt.float8e4,
                                   name=f"bounce_a{i}") for i in range(4)]
            bounces_b = [dram.tile([BSLOTS, HID], dt.float8e4,
                                   name=f"bounce_b{i}") for i in range(4)]
            win_bounce = dram.tile([4 * 128, PG], dt.bfloat16)
            wins_all = dram.tile([NC_ * 4 * 128, PG], dt.bfloat16,
                                 addr_space="Shared")

            # persistent SBUF
            gl_sb = const.tile([128, BLKS * TL * 8], dt.int16)
            gh_sb = const.tile([128, BLKS * TH * 8], dt.int16)
            u_loc = const.tile([128, BLKS * HID], dt.bfloat16)
            cvec = const.tile([1, 5 * 512], dt.bfloat16)
            b2row = const.tile([1, 5 * 512], dt.bfloat16)
            onecol = const.tile([1, 128], dt.bfloat16)
            pvec = const.tile([128, 32], dt.float32)
            ident = const.tile([128, 128], dt.bfloat16)
            wa0 = const.tile([128, F1P], dt.bfloat16)
            wb0 = const.tile([128, 3 * 512], dt.bfloat16)
            wa_sb = [const.tile([128, 4 * 512], dt.bfloat16, name=f"wa{i}")
                     for i in range(4)]
            wb_sb = [const.tile([128, 4 * 512], dt.bfloat16, name=f"wb{i}")
                     for i in range(4)]

            for t, s in ((gl_sb, gl_in), (gh_sb, gh_in),
                         (cvec, cvec_in), (b2row, b2row_in),
                         (onecol, onecol_in), (pvec, pvec_in),
                         (ident, ident_in), (wa0, wa0_in), (wb0, wb0_in)):
                nc.sync.dma_start(out=t[:], in_=s[:])
            for i in range(4):
                nc.sync.dma_start(out=wa_sb[i][:], in_=wa_in[i])
                nc.sync.dma_start(out=wb_sb[i][:], in_=wb_in[i])

            conv_ctx = contextlib.ExitStack()
            m8c = conv_ctx.enter_context(tc.tile_pool(name="m8c", bufs=1))
            gpool = conv_ctx.enter_context(tc.tile_pool(name="gpool", bufs=2))
            aggp = conv_ctx.enter_context(tc.tile_pool(name="aggp", bufs=6))
            h1p = conv_ctx.enter_context(tc.tile_pool(name="h1p", bufs=6))
            u8p = conv_ctx.enter_context(tc.tile_pool(name="u8p", bufs=2))
            degp = conv_ctx.enter_context(tc.tile_pool(name="degp", bufs=2))
            psA = conv_ctx.enter_context(tc.tile_pool(name="psA", bufs=4,
                                                      space="PSUM"))
            psB = conv_ctx.enter_context(tc.tile_pool(name="psB", bufs=2,
                                                      space="PSUM"))
            psC = conv_ctx.enter_context(tc.tile_pool(name="psC", bufs=2,
                                                      space="PSUM"))

            m8_sb = m8c.tile([128, BLKS * NT * 128], dt.float8e4)
            for mc in range(0, BLKS, GRP):   # chunked: early groups unblock
                hi = min(mc + GRP, BLKS)
                nc.sync.dma_start(
                    out=m8_sb[:, mc * NT * 128:hi * NT * 128],
                    in_=m8_in[:, mc * NT * 128:hi * NT * 128])

            def conv_layer(l, pool_a, pool_b, F_in, u_src, dst,
                           pending_ag=None):
                """dst = None or (bounce_a, bounce_b, npool_a, npool_b).
                pending_ag: previous layer's chunk-B AllGather (ins, outs) —
                emitted on the Q7 stream after group 0's A-gather so its
                semaphore wait cannot stall this layer's gather issue."""
                FC = F_in // 128
                F1C = 3 if l == 0 else 4
                wa_t = wa0 if l == 0 else wa_sb[l - 1]
                wb_t = wb0 if l == 0 else wb_sb[l - 1]
                f1w = F1P if l == 0 else 512
                def gather_one(gt, src_ap, idx_sb, tile0, ntiles, qn):
                    done = 0
                    ci = 0
                    while done < ntiles:
                        k = min(8, ntiles - done)
                        nc.gpsimd.dma_gather(
                            out_ap=gt[:, done:done + k, :],
                            in_ap=src_ap,
                            idxs_ap=idx_sb[:, (tile0 + done) * 8:
                                           (tile0 + done + k) * 8],
                            num_idxs=k * 128, num_idxs_reg=k * 128,
                            elem_size=F_in, single_packet=False,
                            queue_num=(qn + 2 * (ci % 2)) % 4)
                        done += k
                        ci += 1

                def new_tiles():
                    g_a = gpool.tile([128, GRP * TL, F_in], dt.float8e4,
                                     tag="ga", bufs=4, name="g_a")
                    g_b = gpool.tile([128, GRP * TH, F_in], dt.float8e4,
                                     tag="gb", bufs=4, name="g_b")
                    return g_a, g_b

                # Layer start (l>=1): batch the first 4 groups' A-gathers
                # BEFORE the pending chunk-B AllGather trigger and all
                # B-gathers, so the in-order Q7 queue drains 4 A-gathers
                # under the collective instead of one.
                handles = {}
                if l > 0:
                    npre = min(4, len(groups))
                    for gj in range(npre):
                        b0j = groups[gj][0]
                        nbj = len(groups[gj])
                        handles[gj] = new_tiles()
                        gather_one(handles[gj][0], pool_a[:], gl_sb,
                                   b0j * TL, nbj * TL, 0)
                    if pending_ag is not None:
                        nc.gpsimd.collective_compute(
                            "AllGather", mybir.AluOpType.bypass,
                            replica_groups=[list(range(NC_))],
                            ins=[pending_ag[0][:]],
                            outs=[pending_ag[1][:]])
                    for gj in range(npre):
                        b0j = groups[gj][0]
                        nbj = len(groups[gj])
                        gather_one(handles[gj][1], pool_b[:], gh_sb,
                                   b0j * TH, nbj * TH, 1)

                for gi, blks in enumerate(groups):
                    nb = len(blks)
                    b0 = blks[0]
                    if gi in handles:
                        g_a, g_b = handles.pop(gi)
                    else:
                        g_a, g_b = new_tiles()
                        if l == 0:
                            nc.sync.dma_start(
                                out=g_a[:, :nb * TL, :],
                                in_=gxa_in[:, b0 * TL * N_FEAT:
                                           (b0 + nb) * TL * N_FEAT]
                                    .rearrange("p (t f) -> p t f", f=F_in))
                            nc.sync.dma_start(
                                out=g_b[:, :nb * TH, :],
                                in_=gxb_in[:, b0 * TH * N_FEAT:
                                           (b0 + nb) * TH * N_FEAT]
                                    .rearrange("p (t f) -> p t f", f=F_in))
                        else:
                            gather_one(g_a, pool_a[:], gl_sb, b0 * TL,
                                       nb * TL, 0)
                            gather_one(g_b, pool_b[:], gh_sb, b0 * TH,
                                       nb * TH, 1)

                    # aggregation into PSUM
                    agg_ps = [psA.tile([128, 512], dt.float32, tag="aggps",
                                       name=f"aggps{fc}", bufs=4)
                              for fc in range(FC)]
                    for bi, b in enumerate(blks):
                        for fc in range(FC):
                            o = agg_ps[fc][:, bi * 128:(bi + 1) * 128]
                            for t in range(TL):
                                nc.tensor.matmul(
                                    out=o,
                                    lhsT=g_a[:, bi * TL + t,
                                             fc * 128:(fc + 1) * 128],
                                    rhs=m8_sb[:, (b * NT + t) * 128:
                                              (b * NT + t + 1) * 128],
                                    start=(t == 0), stop=False)
                            for t in range(TH):
                                nc.tensor.matmul(
                                    out=o,
                                    lhsT=g_b[:, bi * TH + t,
                                             fc * 128:(fc + 1) * 128],
                                    rhs=m8_sb[:, (b * NT + TL + t) * 128:
                                              (b * NT + TL + t + 1) * 128],
                                    start=False, stop=False)
                            nc.tensor.matmul(
                                out=o,
                                lhsT=u_src[:, b * F_in + fc * 128:
                                           b * F_in + (fc + 1) * 128],
                                rhs=ident[:], start=False, stop=True)

                    # evac PSUM -> SBUF bf16 (plain copy; BN folded away)
                    w = nb * 128
                    agg_sb = [aggp.tile([128, 512], dt.bfloat16, tag="agg",
                                        name=f"aggsb{fc}", bufs=8)
                              for fc in range(FC)]
                    for fc in range(FC):
                        nc.vector.tensor_copy(out=agg_sb[fc][:, :w],
                                              in_=agg_ps[fc][:, :w])

                    # dense1 (transposed): h1_T[m] = relu(sum_fc WA.T@agg
                    #                                 + cvec_m * deg + bA_m)
                    deg_g = degp.tile([1, 512], dt.bfloat16, tag="deg",
                                      bufs=2, name="deg_g")
                    nc.sync.dma_start(out=deg_g[0:1, :w],
                                      in_=deg_in[0:1, b0 * 128:b0 * 128 + w])
                    h1_sb = [h1p.tile([128, 512], dt.bfloat16, tag="h1",
                                      name=f"h1sb{m}", bufs=8)
                             for m in range(F1C)]
                    for m in range(F1C):
                        h1_ps = psB.tile([128, 512], dt.float32, tag="h1ps")
                        for fc in range(FC):
                            nc.tensor.matmul(
                                out=h1_ps[:, :w],
                                lhsT=wa_t[:, fc * f1w + m * 128:
                                          fc * f1w + (m + 1) * 128],
                                rhs=agg_sb[fc][:, :w],
                                start=(fc == 0), stop=False)
                        nc.tensor.matmul(
                            out=h1_ps[:, :w],
                            lhsT=cvec[0:1, l * 512 + m * 128:
                                      l * 512 + (m + 1) * 128],
                            rhs=deg_g[0:1, :w],
                            start=False, stop=True)
                        nc.scalar.activation(
                            out=h1_sb[m][:, :w], in_=h1_ps[:, :w],
                            func=mybir.ActivationFunctionType.Relu,
                            bias=pvec[:, l * 4 + m:l * 4 + m + 1])

                    # dense2 per block
                    u8_sb = (u8p.tile([128, GRP * HID], dt.float8e4,
                                      tag="u8", bufs=2, name="u8_sb")
                             if dst is not None else None)
                    for bi, b in enumerate(blks):
                        h2_ps = psC.tile([128, 512], dt.float32, tag="h2ps")
                        for k in range(F1C):
                            nc.tensor.matmul(
                                out=h2_ps[:],
                                lhsT=h1_sb[k][:, bi * 128:(bi + 1) * 128],
                                rhs=wb_t[:, k * 512:(k + 1) * 512],
                                start=(k == 0), stop=False)
                        nc.tensor.matmul(
                            out=h2_ps[:],
                            lhsT=onecol[0:1, :],
                            rhs=b2row[0:1, l * 512:(l + 1) * 512],
                            start=False, stop=True)
                        nc.scalar.activation(
                            out=u_loc[:, b * HID:(b + 1) * HID],
                            in_=h2_ps[:],
                            func=mybir.ActivationFunctionType.Relu)
                        if dst is not None:
                            nc.vector.tensor_scalar(
                                out=u8_sb[:, bi * HID:(bi + 1) * HID],
                                in0=h2_ps[:], scalar1=0.0, scalar2=None,
                                op0=mybir.AluOpType.max)
                    if dst is not None:
                        ba, bb_, _, _ = dst
                        if b0 < ABLKS:
                            nc.sync.dma_start(
                                out=ba[:].rearrange("(p b) f -> p b f",
                                                    b=ABLKS)[:, b0:b0 + nb, :],
                                in_=u8_sb[:, :nb * HID]
                                    .rearrange("p (b f) -> p b f", b=nb))
                        else:
                            nc.sync.dma_start(
                                out=bb_[:].rearrange("(p b) f -> p b f",
                                                     b=BBLKS)[:, b0 - ABLKS:
                                                              b0 - ABLKS + nb, :],
                                in_=u8_sb[:, :nb * HID]
                                    .rearrange("p (b f) -> p b f", b=nb))
                    # AG_A trigger: for l==0 no Q7 gathers exist, emit right
                    # after its inputs; for l>=1 emit late (group 9) so its
                    # bounce-write wait cannot stall gather issue on Q7.
                    if dst is not None and gi == (AG_GROUP if l == 0 else 8):
                        nc.gpsimd.collective_compute(
                            "AllGather", mybir.AluOpType.bypass,
                            replica_groups=[list(range(NC_))],
                            ins=[dst[0][:]], outs=[dst[2][:]])
                # chunk-B AllGather of this layer is emitted by the NEXT
                # layer (pending_ag) for l<4; layer 3's consumer is layer 4.
                return (dst[1], dst[3]) if dst is not None else None

            with tc.tile_pool(name="uxp", bufs=1) as uxp:
                ux = uxp.tile([128, BLKS * N_FEAT], dt.bfloat16)
                nc.sync.dma_start(out=ux[:], in_=ux_in[:])
                with nc.named_scope("layer0"):
                    pend = conv_layer(0, None, None, N_FEAT, ux,
                                      (bounces_a[0], bounces_b[0],
                                       pools_a[0], pools_b[0]))
            for l in range(1, 5):
                dst = ((bounces_a[l], bounces_b[l], pools_a[l], pools_b[l])
                       if l < 4 else None)
                with nc.named_scope(f"layer{l}"):
                    pend = conv_layer(l, pools_a[l - 1], pools_b[l - 1], HID,
                                      u_loc, dst, pending_ag=pend)
            conv_ctx.close()

            # ---------------- pooling into per-core graph window
            with tc.tile_pool(name="pps", bufs=4, space="PSUM") as pps, \
                 tc.tile_pool(name="mpp", bufs=2) as mpp, \
                 tc.tile_pool(name="winp", bufs=1) as winp:
                pool_ps = [pps.tile([128, PG], dt.float32, name=f"poolps{fc}",
                                    tag="poolps", bufs=4)
                           for fc in range(4)]
                for b in range(BLKS):
                    if b % 4 == 0:
                        nbk = min(4, BLKS - b)
                        mp_sb = mpp.tile([128, 4 * PG], dt.bfloat16, tag="mp")
                        nc.sync.dma_start(
                            out=mp_sb[:, :nbk * PG],
                            in_=mp_in[:, b * PG:(b + nbk) * PG])
                    for fc in range(4):
                        nc.tensor.matmul(
                            out=pool_ps[fc][:],
                            lhsT=u_loc[:, b * HID + fc * 128:
                                       b * HID + (fc + 1) * 128],
                            rhs=mp_sb[:, (b % 4) * PG:(b % 4 + 1) * PG],
                            start=(b == 0), stop=(b == BLKS - 1))
                win_sb = winp.tile([128, 4 * PG], dt.bfloat16)
                for fc in range(4):
                    nc.vector.tensor_copy(
                        out=win_sb[:, fc * PG:(fc + 1) * PG],
                        in_=pool_ps[fc][:])
                nc.sync.dma_start(
                    out=win_bounce[:].rearrange("(c p) g -> p c g", p=128),
                    in_=win_sb[:].rearrange("p (c g) -> p c g", c=4))
            nc.gpsimd.collective_compute(
                "AllGather", mybir.AluOpType.bypass,
                replica_groups=[list(range(NC_))],
                ins=[win_bounce[:]], outs=[wins_all[:]])

            # ---------------- reconstruction + head (redundant on all cores)
            with tc.tile_pool(name="headp", bufs=1) as hp, \
                 tc.tile_pool(name="wtmpp", bufs=4) as wtp, \
                 tc.tile_pool(name="hps", bufs=4, space="PSUM") as hps:
                lw_sb = [hp.tile([128, 4 * 512], dt.bfloat16, name=f"lwt{i}")
                         for i in range(3)]
                fw_sb = hp.tile([128, 4], dt.bfloat16)
                for i in range(3):
                    nc.sync.dma_start(out=lw_sb[i][:], in_=lw_in[i])
                nc.sync.dma_start(out=fw_sb[:], in_=fw_in[:])
                pool_full = hp.tile([128, 4 * N_GRAPHS], dt.bfloat16)
                nc.vector.memset(pool_full[:], 0)
                for w in range(NC_):
                    wtmp = wtp.tile([128, 4 * PG], dt.bfloat16, tag="wtmp")
                    nc.sync.dma_start(
                        out=wtmp[:].rearrange("p (c g) -> p c g", c=4),
                        in_=wins_all[w * 512:(w + 1) * 512, :]
                            .rearrange("(c p) g -> p c g", p=128))
                    for fc in range(4):
                        dstv = pool_full[:, fc * N_GRAPHS + WBASES[w]:
                                         fc * N_GRAPHS + WBASES[w] + PG]
                        nc.vector.tensor_add(
                            out=dstv, in0=dstv,
                            in1=wtmp[:, fc * PG:(fc + 1) * PG])

                cur = pool_full
                for li in range(3):
                    nxt = hp.tile([128, 4 * N_GRAPHS], dt.bfloat16,
                                  name=f"head{li}", tag="headbuf", bufs=2)
                    for nk in range(4):
                        for m in range(4):
                            ps = hps.tile([128, 512], dt.float32, tag="hps")
                            for k in range(4):
                                nc.tensor.matmul(
                                    out=ps[:],
                                    lhsT=lw_sb[li][:, k * 512 + m * 128:
                                                   k * 512 + (m + 1) * 128],
                                    rhs=cur[:, k * N_GRAPHS + nk * 512:
                                            k * N_GRAPHS + (nk + 1) * 512],
                                    start=(k == 0), stop=(k == 3))
                            nc.scalar.activation(
                                out=nxt[:, m * N_GRAPHS + nk * 512:
                                        m * N_GRAPHS + (nk + 1) * 512],
                                in_=ps[:],
                                func=mybir.ActivationFunctionType.Relu,
                                bias=pvec[:, 20 + 4 * li + m:
                                          20 + 4 * li + m + 1])
                    cur = nxt
                osb = hp.tile([1, N_GRAPHS], dt.float32)
                for nk in range(4):
                    ps = hps.tile([1, 512], dt.float32, tag="ops")
                    for k in range(4):
                        nc.tensor.matmul(
                            out=ps[:],
                            lhsT=fw_sb[:, k:k + 1],
                            rhs=cur[:, k * N_GRAPHS + nk * 512:
                                    k * N_GRAPHS + (nk + 1) * 512],
                            start=(k == 0), stop=(k == 3))
                    nc.scalar.activation(
                        out=osb[:, nk * 512:(nk + 1) * 512], in_=ps[:],
                        func=mybir.ActivationFunctionType.Copy, bias=FB_CONST)
                nc.sync.dma_start(
                    out=out_ext[:].rearrange("g one -> one g"),
                    in_=osb[:])
    nc.compile()
    return nc


WBASES = None
FB_CONST = 0.0


# ---------------------------------------------------------------- host packing

def make_in_maps(inputs, plan, layers):
    TL, TH = plan["TL"], plan["TH"]
    NT = TL + TH
    slot_of, core_of = plan["slot_of"], plan["core_of"]
    x = np.asarray(inputs["x"], np.float32)

    def wrap_idx(flat):
        n = len(flat)
        arr = flat.reshape(n // 16, 16).T.astype(np.int16)
        return np.tile(arr, (8, 1))

    # v0 = gin0 * x, in both fp8 (pool content for gathers) and bf16 (self)
    v0 = (x * layers[0]["gin"][None, :]).astype(np.float32)
    v0_8 = v0.astype(FP8).astype(np.float32)

    in_maps = []
    for c in range(NC_):
        m = {}
        nodes = np.arange(c * SHARD, (c + 1) * SHARD)
        sl = slot_of[nodes]
        # ux: [128 p, BLKS*128 f] bf16 = v0 of (block, slot)
        uxa = np.zeros((128, BLKS * N_FEAT), np.float32)
        uxa.reshape(128, BLKS, N_FEAT)[sl % 128, sl // 128, :] = v0[nodes]
        m["ux"] = uxa.astype(BF16)

        # pre-gathered layer-0 source rows, fp8, gather-output layout
        # gxa[p, (b*TL+t)*128 + f] = v0_8[src of edge (b, A-tile t, pos p)]
        srcA = np.zeros((BLKS, TL, 128), np.int64)
        srcB = np.zeros((BLKS, TH, 128), np.int64)
        # invert pool rows back to node ids
        rowA_of = np.full(POOL_A, 0, np.int64)
        rowB_of = np.full(POOL_B, 0, np.int64)
        chunk_of = plan["chunk_of"]
        pr = plan["pool_row"]
        selA = chunk_of == 0
        rowA_of[pr[selA]] = np.nonzero(selA)[0]
        selB = chunk_of == 1
        rowB_of[pr[selB]] = np.nonzero(selB)[0]
        srcA = rowA_of[plan["idxA"][c].astype(np.int64)]   # [BLKS, TL, 128]
        srcB = rowB_of[plan["idxB"][c].astype(np.int64)]
        GA = v0_8[srcA]                     # [BLKS, TL, 128, F]
        GB = v0_8[srcB]
        m["gxa"] = np.ascontiguousarray(
            GA.transpose(2, 0, 1, 3).reshape(128, -1)).astype(FP8)
        m["gxb"] = np.ascontiguousarray(
            GB.transpose(2, 0, 1, 3).reshape(128, -1)).astype(FP8)

        m["gl"] = wrap_idx(plan["idxA"][c].reshape(-1))
        m["gh"] = wrap_idx(plan["idxB"][c].reshape(-1))

        mt = plan["M"][c].reshape(BLKS * NT, 128, 128)
        m["m8"] = np.ascontiguousarray(
            mt.transpose(1, 0, 2).reshape(128, -1)).astype(FP8)

        mp = plan["mpool"][c]
        m["mp"] = np.ascontiguousarray(
            mp.transpose(1, 0, 2).reshape(128, -1)).astype(BF16)

        m["deg"] = plan["deg"][c][None, :].astype(BF16)

        cv = np.zeros((1, 5 * 512), np.float32)
        b2 = np.zeros((1, 5 * 512), np.float32)
        for l in range(5):
            cvl = layers[l]["cvec"]
            cv[0, l * 512:l * 512 + len(cvl)] = cvl
            b2[0, l * 512:(l + 1) * 512] = layers[l]["bB"]
        m["cvec"] = cv.astype(BF16)
        m["b2row"] = b2.astype(BF16)
        m["onecol"] = np.ones((1, 128), np.float32).astype(BF16)

        pvec = np.zeros((128, 32), np.float32)
        for l in range(5):
            F1C = 3 if l == 0 else 4
            ba = layers[l]["bA"]
            for mm in range(F1C):
                seg = ba[mm * 128:(mm + 1) * 128]
                pvec[:len(seg), l * 4 + mm] = seg
        for li in range(3):
            lb = np.asarray(inputs["lb"][li], np.float32)
            for mm in range(4):
                pvec[:, 20 + 4 * li + mm] = lb[mm * 128:(mm + 1) * 128]
        m["pvec"] = pvec

        m["ident"] = np.eye(128, dtype=np.float32).astype(BF16)

        wa0 = np.zeros((128, F1P), np.float32)
        wa0[:, :HID1] = layers[0]["WA"]
        m["wa0"] = wa0.astype(BF16)
        wb0 = np.zeros((128, 3 * 512), np.float32)
        WB0 = layers[0]["WB"]
        for k in range(3):
            seg = WB0[k * 128:min((k + 1) * 128, HID1)]
            wb0[:seg.shape[0], k * 512:(k + 1) * 512] = seg
        m["wb0"] = wb0.astype(BF16)

        wa = np.zeros((4, 128, 4 * 512), np.float32)
        wb = np.zeros((4, 128, 4 * 512), np.float32)
        for l in range(1, 5):
            WA, WBm = layers[l]["WA"], layers[l]["WB"]
            for fc in range(4):
                wa[l - 1, :, fc * 512:(fc + 1) * 512] = \
                    WA[fc * 128:(fc + 1) * 128, :]
                wb[l - 1, :, fc * 512:(fc + 1) * 512] = \
                    WBm[fc * 128:(fc + 1) * 128, :]
        m["wa"] = wa.astype(BF16)
        m["wb"] = wb.astype(BF16)

        lw = np.zeros((3, 128, 4 * 512), np.float32)
        for li in range(3):
            LW = np.asarray(inputs["lw"][li], np.float32)
            for k in range(4):
                for mm in range(4):
                    lw[li, :, k * 512 + mm * 128:k * 512 + (mm + 1) * 128] = \
                        LW[k * 128:(k + 1) * 128, mm * 128:(mm + 1) * 128]
        m["lw"] = lw.astype(BF16)

        fw = np.zeros((128, 4), np.float32)
        FW = np.asarray(inputs["fw"], np.float32)
        for k in range(4):
            fw[:, k] = FW[k * 128:(k + 1) * 128, 0]
        m["fw"] = fw.astype(BF16)

        in_maps.append(m)
    return in_maps


_CACHE = {}


def kernel(**inputs):
    global WBASES, FB_CONST
    from concourse.bass_utils import run_bass_kernel_spmd

    plan = build_plan(np.asarray(inputs["edge_index"]),
                      np.asarray(inputs["batch"]))
    layers = fold_params({k: np.asarray(v) for k, v in inputs.items()
                          if k not in ("x", "edge_index", "batch")})
    WBASES = [int(v) for v in plan["wbase"]]
    FB_CONST = float(np.asarray(inputs["fb"]).reshape(-1)[0])

    key = (plan["TL"], plan["TH"], tuple(WBASES), FB_CONST)
    if key not in _CACHE:
        _CACHE[key] = build_device(plan["TL"], plan["TH"])
    nc = _CACHE[key]

    in_maps = make_in_maps(inputs, plan, layers)
    res = run_bass_kernel_spmd(nc, in_maps, core_ids=list(range(NC_)),
                               trace=False)
    out = res.results[0]["out"].astype(np.float32)
    return out
